# revision 28
# baseline (speedup 1.0000x reference)
"""Trainium2 Bass kernel for T5-style relative-position-bias attention.

Problem (hardcoded): B=2, N=2048, H=16, D=64, MODEL=1024
  sim  = q @ k^T per head; sim = (sim + rel_pos_bias) * D**-0.5
  attn = softmax(sim, axis=-1)
  out  = (attn @ v) reshaped to [b, n, MODEL] @ w_out.T + b_out

Sharding: 8 cores = (batch b) x (query-chunk qoff in {0,512,1024,1536}).
Each core computes the full output rows for its 512 queries; no collectives.

Device algorithm per core (transposes all pushed to host):
  S^T[k, q] = kT_h-slices.T @ qT_h   (bf16 matmuls, contraction d=64)
  P = exp(C*S^T) [* E^T on "band" chunks only]
  The bias enters multiplicatively: exp(C*(S+bias)) = exp(C*S)*exp(C*bias).
  T5 buckets saturate for |k-q| >= 128, so chunks of 128 keys that lie
  entirely off the diagonal band have a CONSTANT bias factor per head;
  that constant is folded into host-pre-scaled V' tiles (linearity of PV),
  leaving the elementwise multiply only for <=6 band chunks per core.
  Key chunks are host-permuted into a fixed slot order (const slots 0..9,
  band slots 10..15) so all 8 cores run one identical program.
  No max-subtraction: logits are ~N(0,1) after scaling (safe in fp32).
  O^T[m, q] = sum_k V'[k, m] * P[k, q], V' = [V | ones] (row 64 = denom r).
  Normalize via broadcast 1/r, then y^T = w_out @ O^T + b_out (host
  re-transposes the per-core [1024, 512] outputs).
"""
import sys
import math

sys.path.insert(0, "/opt/trn_rl_repo")

import numpy as np
import ml_dtypes

import concourse.bass as bass
from concourse import bacc
import concourse.tile as tile
from concourse import mybir
from concourse.bass_utils import run_bass_kernel_spmd

F32 = mybir.dt.float32
F32R = mybir.dt.float32r
BF16 = mybir.dt.bfloat16

B, N, H, D = 2, 2048, 16, 64
MODEL = H * D
NQ = 512
NCORES = 8
C = float(D) ** -0.5
NUM_BUCKETS, MAX_DIST = 32, 128
CHUNKS = 16
NBAND = 6                       # band slots 10..15
GROUPS = [(0, 3), (3, 3), (6, 3), (9, 3), (12, 3), (15, 1)]

_CACHE = {}


def _slot_map(qoff):
    """Permutation slot -> original key-chunk j. Band chunks (those touching
    |k-q| < 128 for q in [qoff, qoff+512)) go to slots 10..15; if fewer than
    6 band chunks exist, nearest const chunks fill the extra band slots."""
    j_lo = max(0, -(-(qoff - 254) // 128))          # ceil((qoff-254)/128)
    j_hi = min(CHUNKS - 1, (qoff + 638) // 128)     # floor
    band = list(range(j_lo, j_hi + 1))
    while len(band) < NBAND:                        # pad with neighbors
        if band[0] > 0:
            band.insert(0, band[0] - 1)
        else:
            band.append(band[-1] + 1)
    const = [j for j in range(CHUNKS) if j not in band]
    assert len(band) == NBAND and len(const) == CHUNKS - NBAND
    # band slots occupy 2..7 so both pair head and tail are DVE-free
    return const[:2] + band + const[2:]


def _build_bass():
    nc = bacc.Bacc("TRN2", target_bir_lowering=False, debug=False,
                   num_devices=NCORES)
    qt_d = nc.dram_tensor("qt", [8, 128, NQ], BF16, kind="ExternalInput")
    kt_d = nc.dram_tensor("kt", [8, 128, N], BF16, kind="ExternalInput")
    vv_d = nc.dram_tensor("vv", [H, 128, CHUNKS, D + 1], BF16, kind="ExternalInput")
    bt_d = nc.dram_tensor("bt", [H, 128, NBAND * NQ], BF16, kind="ExternalInput")
    wt_d = nc.dram_tensor("wt", [8, 128, MODEL], BF16, kind="ExternalInput")
    bv_d = nc.dram_tensor("bv", [128, 8], F32, kind="ExternalInput")
    yt_d = nc.dram_tensor("yt", [8, 128, NQ], F32, kind="ExternalOutput")

    with tile.TileContext(nc) as tc:
        with tc.tile_pool(name="const", bufs=1) as cpool:
            qt_ts = []
            for hp in range(8):
                t = cpool.tile([128, NQ], BF16, tag=f"qt{hp}", name=f"qt{hp}")
                qt_ts.append(t)
            nc.sync.dma_start(qt_ts[0], qt_d[0])
            wt_ts = []
            bv_t = None
            ocat_ts = [cpool.tile([128, NQ], BF16, tag=f"ocat{mc}",
                                  name=f"ocat{mc}")
                       for mc in range(8)]

            with tc.tile_pool(name="kt", bufs=2) as ktpool, \
                 tc.tile_pool(name="vv", bufs=4) as vvpool, \
                 tc.tile_pool(name="bt", bufs=4) as btpool, \
                 tc.tile_pool(name="p0", bufs=8) as p0pool, \
                 tc.tile_pool(name="pm", bufs=12) as pmpool, \
                 tc.tile_pool(name="sm", bufs=6) as smpool, \
                 tc.tile_pool(name="stps", bufs=2, space="PSUM") as stp, \
                 tc.tile_pool(name="ops", bufs=2, space="PSUM") as opool:
                # Flat software-pipelined stream over all pairs' tiles.
                # Each tile: 3 units of [128, 512] where unit = (slot, head).
                # QK matmuls of h0 (PE rows 0-63) and h1 (rows 64-127) are
                # adjacent so hardware runs them concurrently in disjoint row
                # groups; exp covers a whole tile in one ACT call. PV matmuls
                # are emitted DELAY tiles late so the next pair's QK bridges
                # the pair boundary on the PE instead of idling the ACT.
                DELAY = 4
                units_per_pair = [(s, h01) for s in range(CHUNKS)
                                  for h01 in range(2)]
                tiles = []
                for hp in range(8):
                    for u0 in range(0, len(units_per_pair), 3):
                        tiles.append((hp, units_per_pair[u0:u0 + 3],
                                      u0 == 0,
                                      u0 + 3 >= len(units_per_pair)))
                state = {}      # hp -> (kt_t, vv_ts, bt_ts, o_pss)
                pv_queue = []   # (hp, chunk, p0, pms)

                def emit_pv(hp, chunk, p0, pms):
                    kt_t, vv_ts, bt_ts, o_pss = state[hp]
                    for uu, (s, h01) in enumerate(chunk):
                        src = pms[uu] if uu in pms else p0[:, uu * NQ:(uu + 1) * NQ]
                        nc.tensor.matmul(
                            o_pss[h01], vv_ts[h01][:, s, :], src,
                            start=(s == 0), stop=(s == CHUNKS - 1))

                def emit_normalize(hp):
                    _, _, _, o_pss = state[hp]
                    order = (1, 0) if hp == 7 else (0, 1)
                    for h01 in order:
                        o_ps = o_pss[h01]
                        rstage = smpool.tile([1, NQ], F32, tag="rstage",
                                             name="rstage")
                        nc.vector.tensor_copy(rstage[0:1, :], o_ps[64:65, :])
                        ocopy = smpool.tile([64, NQ], F32, tag="ocopy",
                                            name="ocopy")
                        nc.vector.tensor_copy(ocopy, o_ps[0:64, :])
                        rbr = smpool.tile([128, NQ], F32, tag="rbr",
                                          name="rbr")
                        nc.gpsimd.partition_broadcast(rbr, rstage[0:1, :])
                        rb = smpool.tile([128, NQ], F32, tag="rb", name="rb")
                        nc.vector.reciprocal_approx_fast(rb, rbr)
                        if h01 == 0:
                            nc.vector.tensor_tensor(
                                ocat_ts[hp][0:64, :], ocopy,
                                rb[0:64, :], mybir.AluOpType.mult)
                        else:
                            s64 = smpool.tile([64, NQ], BF16, tag="s64",
                                              name="s64")
                            nc.vector.tensor_tensor(
                                s64, ocopy, rb[0:64, :],
                                mybir.AluOpType.mult)
                            nc.sync.dma_start(ocat_ts[hp][64:128, :], s64)

                for hp, chunk, is_first, is_last in tiles:
                    if is_first:
                        if hp == 1:
                            for mc in range(8):
                                t = cpool.tile([128, MODEL], BF16,
                                               tag=f"wt{mc}", name=f"wt{mc}")
                                nc.sync.dma_start(t, wt_d[mc])
                                wt_ts.append(t)
                            bv_t = cpool.tile([128, 8], F32, tag="bv",
                                              name="bv")
                            nc.sync.dma_start(bv_t, bv_d[:, :])
                        kt_t = ktpool.tile([128, N], BF16, tag="kt", name="kt")
                        if hp == 0:
                            nc.sync.dma_start(kt_t[:, :256], kt_d[hp][:, :256])
                            nc.sync.dma_start(kt_t[:, 256:], kt_d[hp][:, 256:])
                        else:
                            nc.sync.dma_start(kt_t, kt_d[hp])
                        if hp + 1 < 8:
                            nc.sync.dma_start(qt_ts[hp + 1], qt_d[hp + 1])
                        vv_ts, bt_ts, o_pss = [], [], []
                        for h01 in range(2):
                            h = 2 * hp + h01
                            vv_t = vvpool.tile([128, CHUNKS, D + 1], BF16,
                                               tag="vv", name="vv")
                            nc.sync.dma_start(vv_t, vv_d[h])
                            bt_t = btpool.tile([128, NBAND, NQ], BF16,
                                               tag="bt", name="bt")
                            nc.sync.dma_start(
                                bt_t,
                                bt_d[h].rearrange("p (s f) -> p s f", s=NBAND))
                            vv_ts.append(vv_t)
                            bt_ts.append(bt_t)
                            o_pss.append(opool.tile([D + 1, NQ], F32,
                                                    tag="ops", name="ops"))
                        state[hp] = (kt_t, vv_ts, bt_ts, o_pss)
                    kt_t, vv_ts, bt_ts, o_pss = state[hp]
                    gw = len(chunk) * NQ
                    st = stp.tile([128, 3 * NQ], F32, tag="st", name="st")
                    for uu, (s, h01) in enumerate(chunk):
                        lo, hi = h01 * 64, h01 * 64 + 64
                        nc.tensor.matmul(
                            st[:, uu * NQ:(uu + 1) * NQ],
                            kt_t[lo:hi, s * 128:(s + 1) * 128],
                            qt_ts[hp][lo:hi, :],
                            start=True, stop=True)
                    p0 = p0pool.tile([128, 3 * NQ], BF16, tag="p0", name="p0")
                    nc.scalar.activation(
                        p0[:, :gw], st[:, :gw],
                        mybir.ActivationFunctionType.Exp, bias=0.0, scale=C)
                    pms = {}
                    for uu, (s, h01) in enumerate(chunk):
                        if 2 <= s < 2 + NBAND:       # band slot: bias factor
                            pm = pmpool.tile([128, NQ], BF16, tag="pm",
                                             name="pm")
                            nc.vector.tensor_tensor(
                                pm, p0[:, uu * NQ:(uu + 1) * NQ],
                                bt_ts[h01][:, s - 2, :],
                                mybir.AluOpType.mult)
                            pms[uu] = pm
                    pv_queue.append((hp, chunk, p0, pms, is_last))
                    if len(pv_queue) > DELAY:
                        qhp, qchunk, qp0, qpms, qlast = pv_queue.pop(0)
                        emit_pv(qhp, qchunk, qp0, qpms)
                        if qlast:
                            emit_normalize(qhp)
                for qhp, qchunk, qp0, qpms, qlast in pv_queue:
                    emit_pv(qhp, qchunk, qp0, qpms)
                    if qlast:
                        emit_normalize(qhp)

            with tc.tile_pool(name="ysb", bufs=2) as ypool, \
                 tc.tile_pool(name="fin", bufs=8, space="PSUM") as fpool:
                for oc in range(8):
                    fp = fpool.tile([128, NQ], F32, tag="fp", name="fp")
                    for mc in range(8):
                        nc.tensor.matmul(
                            fp, wt_ts[mc][:, oc * 128:(oc + 1) * 128],
                            ocat_ts[mc], start=(mc == 0), stop=(mc == 7))
                    ysb = ypool.tile([128, NQ], F32, tag="ysb", name="ysb")
                    nc.scalar.add(ysb, fp, bv_t[:, oc:oc + 1])
                    nc.sync.dma_start(yt_d[oc], ysb)
    nc.compile()
    return nc


def _rel_pos_bucket_np(rel):
    """T5 bidirectional bucketing, float32 math mirroring the jnp reference."""
    nb = NUM_BUCKETS // 2
    ret = (rel >= 0).astype(np.int32) * nb
    n = np.abs(rel)
    max_exact = nb // 2
    is_small = n < max_exact
    n_safe = np.maximum(n, 1).astype(np.float32)
    val_large = max_exact + (
        np.log(n_safe / np.float32(max_exact)).astype(np.float32)
        / np.float32(math.log(MAX_DIST / max_exact)) * np.float32(nb - max_exact)
    ).astype(np.int32)
    val_large = np.minimum(val_large, nb - 1)
    return ret + np.where(is_small, n, val_large)


def _e_diag(rel_emb):
    """e_diag[h, r + 2047] = exp(C * rel_emb[bucket(r), h]), r in [-2047, 2047]."""
    rel = np.arange(-2047, 2048, dtype=np.int32)
    buckets = _rel_pos_bucket_np(rel)
    e = np.exp(np.float32(C) * np.asarray(rel_emb, np.float32)[buckets, :])
    return np.ascontiguousarray(e.T)                 # [H, 4095]


def _prep_inputs(q, k, v, rel_emb, w_out, b_out):
    q = np.asarray(q, np.float32)
    k = np.asarray(k, np.float32)
    v = np.asarray(v, np.float32)
    rel_emb = np.asarray(rel_emb, np.float32)
    ediag = _e_diag(rel_emb)
    # constant bias factors of the two saturated bucket regions, per head
    e_pos = np.exp(np.float32(C) * rel_emb[31, :])   # k - q >= 128
    e_neg = np.exp(np.float32(C) * rel_emb[15, :])   # k - q <= -128
    wt = np.ascontiguousarray(np.asarray(w_out, np.float32).T).reshape(8, 128, MODEL)
    bv = np.ascontiguousarray(np.asarray(b_out, np.float32).reshape(8, 128).T)
    p = np.arange(128)
    u = np.arange(NQ)
    in_maps = []
    for core in range(NCORES):
        b, qc = divmod(core, 4)
        qoff = qc * NQ
        smap = _slot_map(qoff)                       # slot -> chunk j
        qs = q[b, qoff:qoff + NQ].reshape(NQ, 8, 2, 64)
        qt = np.ascontiguousarray(qs.transpose(1, 2, 3, 0)).reshape(8, 128, NQ)
        kt = np.ascontiguousarray(
            k[b].reshape(N, 8, 2, 64).transpose(1, 2, 3, 0)).reshape(8, 128, N)
        kt = np.ascontiguousarray(
            kt.reshape(8, 128, CHUNKS, 128)[:, :, smap, :]).reshape(8, 128, N)
        vs = v[b].reshape(CHUNKS, 128, H, D).transpose(2, 1, 0, 3)  # [h,kk,j,d]
        vv = np.concatenate(
            [vs, np.ones((H, 128, CHUNKS, 1), np.float32)], axis=-1)
        vv = vv[:, :, smap, :]                       # slot order
        # scale const slots by their constant bias factor; band slots (2..7)
        # get the full bias factor from bt instead, even if saturated
        for s in list(range(2)) + list(range(2 + NBAND, CHUNKS)):
            j = smap[s]
            rel_min = 128 * j - qoff - (NQ - 1)      # min over tile of k - q
            rel_max = 128 * j + 127 - qoff
            if rel_min >= 128:
                fac = e_pos
            elif rel_max <= -128:
                fac = e_neg
            else:
                raise AssertionError(
                    f"band chunk {j} in const slot {s} (qoff={qoff})")
            vv[:, :, s, :] *= fac[:, None, None]
        bt = np.empty((H, 128, NBAND, NQ), np.float32)
        for sb in range(NBAND):
            j = smap[2 + sb]
            idx = (128 * j + p[:, None]) - (qoff + u[None, :]) + 2047
            bt[:, :, sb, :] = ediag[:, idx]
        in_maps.append({
            "qt": qt.astype(ml_dtypes.bfloat16),
            "kt": kt.astype(ml_dtypes.bfloat16),
            "vv": np.ascontiguousarray(vv).astype(ml_dtypes.bfloat16),
            "bt": np.ascontiguousarray(
                bt.reshape(H, 128, NBAND * NQ).astype(ml_dtypes.bfloat16)),
            "wt": wt.astype(ml_dtypes.bfloat16), "bv": bv,
        })
    return in_maps


def _run(q, k, v, rel_emb, w_out, b_out, trace=False):
    if "nc" not in _CACHE:
        _CACHE["nc"] = _build_bass()
    nc = _CACHE["nc"]
    in_maps = _prep_inputs(q, k, v, rel_emb, w_out, b_out)
    res = run_bass_kernel_spmd(nc, in_maps, core_ids=list(range(NCORES)),
                               trace=trace)
    y = np.empty((B, N, MODEL), np.float32)
    for core in range(NCORES):
        b, qc = divmod(core, 4)
        qoff = qc * NQ
        yt = res.results[core]["yt"]
        y[b, qoff:qoff + NQ] = yt.transpose(2, 0, 1).reshape(NQ, MODEL)
    return y, res


def kernel(q, k, v, rel_emb, w_out, b_out):
    y, _ = _run(q, k, v, rel_emb, w_out, b_out, trace=False)
    return y


# revision 29
# speedup vs baseline: 1.0565x; 1.0565x over previous
"""Trainium2 Bass kernel for T5-style relative-position-bias attention.

Problem (hardcoded): B=2, N=2048, H=16, D=64, MODEL=1024
  sim  = q @ k^T per head; sim = (sim + rel_pos_bias) * D**-0.5
  attn = softmax(sim, axis=-1)
  out  = (attn @ v) reshaped to [b, n, MODEL] @ w_out.T + b_out

Sharding: 8 cores = (batch b) x (query-chunk qoff in {0,512,1024,1536}).
Each core computes the full output rows for its 512 queries; no collectives.

Device algorithm per core (transposes all pushed to host):
  S^T[k, q] = kT_h-slices.T @ qT_h   (bf16 matmuls, contraction d=64)
  P = exp(C*S^T) [* E^T on "band" chunks only]
  The bias enters multiplicatively: exp(C*(S+bias)) = exp(C*S)*exp(C*bias).
  T5 buckets saturate for |k-q| >= 128, so chunks of 128 keys that lie
  entirely off the diagonal band have a CONSTANT bias factor per head;
  that constant is folded into host-pre-scaled V' tiles (linearity of PV),
  leaving the elementwise multiply only for <=6 band chunks per core.
  Key chunks are host-permuted into a fixed slot order (const slots 0..9,
  band slots 10..15) so all 8 cores run one identical program.
  No max-subtraction: logits are ~N(0,1) after scaling (safe in fp32).
  O^T[m, q] = sum_k V'[k, m] * P[k, q], V' = [V | ones] (row 64 = denom r).
  Normalize via broadcast 1/r, then y^T = w_out @ O^T + b_out (host
  re-transposes the per-core [1024, 512] outputs).
"""
import sys
import math

sys.path.insert(0, "/opt/trn_rl_repo")

import numpy as np
import ml_dtypes

import concourse.bass as bass
from concourse import bacc
import concourse.tile as tile
from concourse import mybir
from concourse.bass_utils import run_bass_kernel_spmd

F32 = mybir.dt.float32
F32R = mybir.dt.float32r
BF16 = mybir.dt.bfloat16

B, N, H, D = 2, 2048, 16, 64
MODEL = H * D
NQ = 512
NCORES = 8
C = float(D) ** -0.5
NUM_BUCKETS, MAX_DIST = 32, 128
CHUNKS = 16
NBAND = 6                       # band slots 10..15
GROUPS = [(0, 3), (3, 3), (6, 3), (9, 3), (12, 3), (15, 1)]

_CACHE = {}


def _slot_map(qoff):
    """Permutation slot -> original key-chunk j. Band chunks (those touching
    |k-q| < 128 for q in [qoff, qoff+512)) go to slots 10..15; if fewer than
    6 band chunks exist, nearest const chunks fill the extra band slots."""
    j_lo = max(0, -(-(qoff - 254) // 128))          # ceil((qoff-254)/128)
    j_hi = min(CHUNKS - 1, (qoff + 638) // 128)     # floor
    band = list(range(j_lo, j_hi + 1))
    while len(band) < NBAND:                        # pad with neighbors
        if band[0] > 0:
            band.insert(0, band[0] - 1)
        else:
            band.append(band[-1] + 1)
    const = [j for j in range(CHUNKS) if j not in band]
    assert len(band) == NBAND and len(const) == CHUNKS - NBAND
    return const + band                             # slots 0..9 const, 10..15 band


def _build_bass():
    nc = bacc.Bacc("TRN2", target_bir_lowering=False, debug=False,
                   num_devices=NCORES)
    qt_d = nc.dram_tensor("qt", [8, 128, NQ], BF16, kind="ExternalInput")
    kt_d = nc.dram_tensor("kt", [8, 128, N], BF16, kind="ExternalInput")
    vv_d = nc.dram_tensor("vv", [H, 128, CHUNKS, D + 1], BF16, kind="ExternalInput")
    bt_d = nc.dram_tensor("bt", [H, 128, NBAND * NQ], BF16, kind="ExternalInput")
    wt_d = nc.dram_tensor("wt", [8, 128, MODEL], BF16, kind="ExternalInput")
    bv_d = nc.dram_tensor("bv", [128, 8], F32, kind="ExternalInput")
    yt_d = nc.dram_tensor("yt", [8, 128, NQ], F32, kind="ExternalOutput")

    with tile.TileContext(nc) as tc:
        with tc.tile_pool(name="const", bufs=1) as cpool:
            qt_ts = []
            for hp in range(8):
                t = cpool.tile([128, NQ], BF16, tag=f"qt{hp}", name=f"qt{hp}")
                qt_ts.append(t)
            nc.sync.dma_start(qt_ts[0], qt_d[0])
            wt_ts = []
            bv_t = None
            ocat_ts = [cpool.tile([128, NQ], BF16, tag=f"ocat{mc}",
                                  name=f"ocat{mc}")
                       for mc in range(8)]

            with tc.tile_pool(name="kt", bufs=2) as ktpool, \
                 tc.tile_pool(name="vv", bufs=4) as vvpool, \
                 tc.tile_pool(name="bt", bufs=4) as btpool, \
                 tc.tile_pool(name="p0", bufs=8) as p0pool, \
                 tc.tile_pool(name="pm", bufs=12) as pmpool, \
                 tc.tile_pool(name="sm", bufs=6) as smpool, \
                 tc.tile_pool(name="stps", bufs=2, space="PSUM") as stp, \
                 tc.tile_pool(name="ops", bufs=2, space="PSUM") as opool:
                # Flat software-pipelined stream over all pairs' tiles.
                # Each tile: 3 units of [128, 512] where unit = (slot, head).
                # QK matmuls of h0 (PE rows 0-63) and h1 (rows 64-127) are
                # adjacent so hardware runs them concurrently in disjoint row
                # groups; exp covers a whole tile in one ACT call. PV matmuls
                # are emitted DELAY tiles late so the next pair's QK bridges
                # the pair boundary on the PE instead of idling the ACT.
                DELAY = 4
                units_per_pair = [(s, h01) for s in range(CHUNKS)
                                  for h01 in range(2)]
                tiles = []
                for hp in range(8):
                    for u0 in range(0, len(units_per_pair), 3):
                        tiles.append((hp, units_per_pair[u0:u0 + 3],
                                      u0 == 0,
                                      u0 + 3 >= len(units_per_pair)))
                state = {}      # hp -> (kt_t, vv_ts, bt_ts, o_pss)
                pv_queue = []   # (hp, chunk, p0, pms)

                def emit_pv(hp, chunk, p0, pms):
                    kt_t, vv_ts, bt_ts, o_pss = state[hp]
                    for uu, (s, h01) in enumerate(chunk):
                        src = pms[uu] if uu in pms else p0[:, uu * NQ:(uu + 1) * NQ]
                        nc.tensor.matmul(
                            o_pss[h01], vv_ts[h01][:, s, :], src,
                            start=(s == 0), stop=(s == CHUNKS - 1))

                def emit_normalize(hp):
                    _, _, _, o_pss = state[hp]
                    for h01 in range(2):
                        o_ps = o_pss[h01]
                        rstage = smpool.tile([1, NQ], F32, tag="rstage",
                                             name="rstage")
                        nc.vector.tensor_copy(rstage[0:1, :], o_ps[64:65, :])
                        ocopy = smpool.tile([64, NQ], F32, tag="ocopy",
                                            name="ocopy")
                        nc.vector.tensor_copy(ocopy, o_ps[0:64, :])
                        rbr = smpool.tile([128, NQ], F32, tag="rbr",
                                          name="rbr")
                        nc.gpsimd.partition_broadcast(rbr, rstage[0:1, :])
                        rb = smpool.tile([128, NQ], F32, tag="rb", name="rb")
                        nc.vector.reciprocal_approx_fast(rb, rbr)
                        if h01 == 0:
                            nc.vector.tensor_tensor(
                                ocat_ts[hp][0:64, :], ocopy,
                                rb[0:64, :], mybir.AluOpType.mult)
                        else:
                            s64 = smpool.tile([64, NQ], BF16, tag="s64",
                                              name="s64")
                            nc.vector.tensor_tensor(
                                s64, ocopy, rb[0:64, :],
                                mybir.AluOpType.mult)
                            nc.sync.dma_start(ocat_ts[hp][64:128, :], s64)

                for hp, chunk, is_first, is_last in tiles:
                    if is_first:
                        if hp == 1:
                            for mc in range(8):
                                t = cpool.tile([128, MODEL], BF16,
                                               tag=f"wt{mc}", name=f"wt{mc}")
                                nc.sync.dma_start(t, wt_d[mc])
                                wt_ts.append(t)
                            bv_t = cpool.tile([128, 8], F32, tag="bv",
                                              name="bv")
                            nc.sync.dma_start(bv_t, bv_d[:, :])
                        kt_t = ktpool.tile([128, N], BF16, tag="kt", name="kt")
                        if hp == 0:
                            nc.sync.dma_start(kt_t[:, :256], kt_d[hp][:, :256])
                            nc.sync.dma_start(kt_t[:, 256:], kt_d[hp][:, 256:])
                        else:
                            nc.sync.dma_start(kt_t, kt_d[hp])
                        if hp + 1 < 8:
                            nc.sync.dma_start(qt_ts[hp + 1], qt_d[hp + 1])
                        vv_ts, bt_ts, o_pss = [], [], []
                        for h01 in range(2):
                            h = 2 * hp + h01
                            vv_t = vvpool.tile([128, CHUNKS, D + 1], BF16,
                                               tag="vv", name="vv")
                            nc.sync.dma_start(vv_t, vv_d[h])
                            bt_t = btpool.tile([128, NBAND, NQ], BF16,
                                               tag="bt", name="bt")
                            nc.sync.dma_start(
                                bt_t,
                                bt_d[h].rearrange("p (s f) -> p s f", s=NBAND))
                            vv_ts.append(vv_t)
                            bt_ts.append(bt_t)
                            o_pss.append(opool.tile([D + 1, NQ], F32,
                                                    tag="ops", name="ops"))
                        state[hp] = (kt_t, vv_ts, bt_ts, o_pss)
                    kt_t, vv_ts, bt_ts, o_pss = state[hp]
                    gw = len(chunk) * NQ
                    st = stp.tile([128, 3 * NQ], F32, tag="st", name="st")
                    for uu, (s, h01) in enumerate(chunk):
                        lo, hi = h01 * 64, h01 * 64 + 64
                        nc.tensor.matmul(
                            st[:, uu * NQ:(uu + 1) * NQ],
                            kt_t[lo:hi, s * 128:(s + 1) * 128],
                            qt_ts[hp][lo:hi, :],
                            start=True, stop=True)
                    p0 = p0pool.tile([128, 3 * NQ], BF16, tag="p0", name="p0")
                    nc.scalar.activation(
                        p0[:, :gw], st[:, :gw],
                        mybir.ActivationFunctionType.Exp, bias=0.0, scale=C)
                    pms = {}
                    for uu, (s, h01) in enumerate(chunk):
                        if s >= CHUNKS - NBAND:      # band slot: bias factor
                            pm = pmpool.tile([128, NQ], BF16, tag="pm",
                                             name="pm")
                            nc.vector.tensor_tensor(
                                pm, p0[:, uu * NQ:(uu + 1) * NQ],
                                bt_ts[h01][:, s - (CHUNKS - NBAND), :],
                                mybir.AluOpType.mult)
                            pms[uu] = pm
                    pv_queue.append((hp, chunk, p0, pms, is_last))
                    if len(pv_queue) > DELAY:
                        qhp, qchunk, qp0, qpms, qlast = pv_queue.pop(0)
                        emit_pv(qhp, qchunk, qp0, qpms)
                        if qlast:
                            emit_normalize(qhp)
                for qhp, qchunk, qp0, qpms, qlast in pv_queue:
                    emit_pv(qhp, qchunk, qp0, qpms)
                    if qlast:
                        emit_normalize(qhp)

            with tc.tile_pool(name="ysb", bufs=2) as ypool, \
                 tc.tile_pool(name="fin", bufs=4, space="PSUM") as fpool:
                for oc in range(8):
                    fp = fpool.tile([128, NQ], F32, tag="fp", name="fp")
                    for mc in range(8):
                        nc.tensor.matmul(
                            fp, wt_ts[mc][:, oc * 128:(oc + 1) * 128],
                            ocat_ts[mc], start=(mc == 0), stop=(mc == 7))
                    ysb = ypool.tile([128, NQ], F32, tag="ysb", name="ysb")
                    nc.scalar.add(ysb, fp, bv_t[:, oc:oc + 1])
                    nc.sync.dma_start(yt_d[oc], ysb)
    nc.compile()
    return nc


def _rel_pos_bucket_np(rel):
    """T5 bidirectional bucketing, float32 math mirroring the jnp reference."""
    nb = NUM_BUCKETS // 2
    ret = (rel >= 0).astype(np.int32) * nb
    n = np.abs(rel)
    max_exact = nb // 2
    is_small = n < max_exact
    n_safe = np.maximum(n, 1).astype(np.float32)
    val_large = max_exact + (
        np.log(n_safe / np.float32(max_exact)).astype(np.float32)
        / np.float32(math.log(MAX_DIST / max_exact)) * np.float32(nb - max_exact)
    ).astype(np.int32)
    val_large = np.minimum(val_large, nb - 1)
    return ret + np.where(is_small, n, val_large)


def _e_diag(rel_emb):
    """e_diag[h, r + 2047] = exp(C * rel_emb[bucket(r), h]), r in [-2047, 2047]."""
    rel = np.arange(-2047, 2048, dtype=np.int32)
    buckets = _rel_pos_bucket_np(rel)
    e = np.exp(np.float32(C) * np.asarray(rel_emb, np.float32)[buckets, :])
    return np.ascontiguousarray(e.T)                 # [H, 4095]


def _prep_inputs(q, k, v, rel_emb, w_out, b_out):
    q = np.asarray(q, np.float32)
    k = np.asarray(k, np.float32)
    v = np.asarray(v, np.float32)
    rel_emb = np.asarray(rel_emb, np.float32)
    ediag = _e_diag(rel_emb)
    # constant bias factors of the two saturated bucket regions, per head
    e_pos = np.exp(np.float32(C) * rel_emb[31, :])   # k - q >= 128
    e_neg = np.exp(np.float32(C) * rel_emb[15, :])   # k - q <= -128
    wt = np.ascontiguousarray(np.asarray(w_out, np.float32).T).reshape(8, 128, MODEL)
    bv = np.ascontiguousarray(np.asarray(b_out, np.float32).reshape(8, 128).T)
    p = np.arange(128)
    u = np.arange(NQ)
    in_maps = []
    for core in range(NCORES):
        b, qc = divmod(core, 4)
        qoff = qc * NQ
        smap = _slot_map(qoff)                       # slot -> chunk j
        qs = q[b, qoff:qoff + NQ].reshape(NQ, 8, 2, 64)
        qt = np.ascontiguousarray(qs.transpose(1, 2, 3, 0)).reshape(8, 128, NQ)
        kt = np.ascontiguousarray(
            k[b].reshape(N, 8, 2, 64).transpose(1, 2, 3, 0)).reshape(8, 128, N)
        kt = np.ascontiguousarray(
            kt.reshape(8, 128, CHUNKS, 128)[:, :, smap, :]).reshape(8, 128, N)
        vs = v[b].reshape(CHUNKS, 128, H, D).transpose(2, 1, 0, 3)  # [h,kk,j,d]
        vv = np.concatenate(
            [vs, np.ones((H, 128, CHUNKS, 1), np.float32)], axis=-1)
        vv = vv[:, :, smap, :]                       # slot order
        # scale const slots by their constant bias factor; band slots (>=10)
        # get the full bias factor from bt instead, even if saturated
        for s in range(CHUNKS - NBAND):
            j = smap[s]
            rel_min = 128 * j - qoff - (NQ - 1)      # min over tile of k - q
            rel_max = 128 * j + 127 - qoff
            if rel_min >= 128:
                fac = e_pos
            elif rel_max <= -128:
                fac = e_neg
            else:
                raise AssertionError(
                    f"band chunk {j} in const slot {s} (qoff={qoff})")
            vv[:, :, s, :] *= fac[:, None, None]
        bt = np.empty((H, 128, NBAND, NQ), np.float32)
        for sb in range(NBAND):
            j = smap[10 + sb]
            idx = (128 * j + p[:, None]) - (qoff + u[None, :]) + 2047
            bt[:, :, sb, :] = ediag[:, idx]
        in_maps.append({
            "qt": qt.astype(ml_dtypes.bfloat16),
            "kt": kt.astype(ml_dtypes.bfloat16),
            "vv": np.ascontiguousarray(vv).astype(ml_dtypes.bfloat16),
            "bt": np.ascontiguousarray(
                bt.reshape(H, 128, NBAND * NQ).astype(ml_dtypes.bfloat16)),
            "wt": wt.astype(ml_dtypes.bfloat16), "bv": bv,
        })
    return in_maps


def _run(q, k, v, rel_emb, w_out, b_out, trace=False):
    if "nc" not in _CACHE:
        _CACHE["nc"] = _build_bass()
    nc = _CACHE["nc"]
    in_maps = _prep_inputs(q, k, v, rel_emb, w_out, b_out)
    res = run_bass_kernel_spmd(nc, in_maps, core_ids=list(range(NCORES)),
                               trace=trace)
    y = np.empty((B, N, MODEL), np.float32)
    for core in range(NCORES):
        b, qc = divmod(core, 4)
        qoff = qc * NQ
        yt = res.results[core]["yt"]
        y[b, qoff:qoff + NQ] = yt.transpose(2, 0, 1).reshape(NQ, MODEL)
    return y, res


def kernel(q, k, v, rel_emb, w_out, b_out):
    y, _ = _run(q, k, v, rel_emb, w_out, b_out, trace=False)
    return y
